# revision 20
# baseline (speedup 1.0000x reference)
"""2-layer heterogeneous GAT (word-word + word-doc relations) on 8 TRN2 cores.

Strategy (edge-parallel, src-sharded):
  - Words/docs row-sharded across 8 cores. Edges assigned to core(src//SH),
    sorted by destination. Each core computes z = x@W for its node rows only.
  - Per-edge attention logits via "fat-row" (256B) dma_gather from local
    el-tables and AllGathered er-tables.
  - Softmax denominators s: per-edge ex scattered (dma_scatter_add) into
    compact per-destination-owner rows, AllToAll'd to owners, merged,
    reciprocal'd, AllGathered back as a fat inv_s table.
  - Messages: gather z rows (2KB) by src (local table), weight by
    alpha = ex*inv_s on DVE with per-head fused mul-adds (head sum folded in),
    scatter-add into compact per-owner rows, AllToAll, owner merges into its
    dense output shard. Bias+relu fused into next layer's load.
  - Outputs are per-core shards; host concatenates. No full-size AllReduduce.
"""

import numpy as np

P = 128
NCORES = 8
H = 4
F = 128
HF = 512
FATW = 64          # fat-row width (f32 words) for 256B-row gather tables
BUCKET = 32768     # int16 index range per dma_gather bucket
DUMP = 2048        # dump rows appended to owner shards for scatter pads


def _pad_up(x, m):
    return (x + m - 1) // m * m


def _wrap16(idx, dtype=np.int16):
    """Layout an index vector the way dma_gather/dma_scatter_add expect:
    [16, n/16] with element i at [i % 16, i // 16]."""
    n = len(idx)
    assert n % 16 == 0
    w = np.ascontiguousarray(idx.reshape(n // 16, 16).T).astype(dtype)
    return np.tile(w, (8, 1))  # replicated across the 8 GPSIMD cores


def _edge_layout(arr, dtype=np.float32):
    """Per-edge array -> [128, ntiles] with edge i at [i % 128, i // 128]."""
    n = len(arr)
    assert n % 128 == 0
    return np.ascontiguousarray(arr.reshape(n // 128, 128).T).astype(dtype)


class RelPrep:
    """Host-side index prep for one relation (shared by both layers)."""

    def __init__(self, src, dst, n_src, n_dst, src_sh, dst_sh, src_pad, dst_pad,
                 chunk_tiles=8):
        self.src_sh, self.dst_sh = src_sh, dst_sh
        self.src_pad, self.dst_pad = src_pad, dst_pad
        self.CH = chunk_tiles

        core_of = src // src_sh
        fatrow = (dst // dst_sh) * dst_pad + dst % dst_sh

        # Per-core edge stream sorted by (owner, dst) == fatrow, packed into
        # 128-edge tiles with segment alignment (no dst straddles a tile).
        # Chunks of <=CH tiles never cross bucket or owner boundaries.
        # Compact/send space: one 128-row group per tile (row = tile*128 +
        # tile-local unique-dst slot). Owner blocks are contiguous because the
        # schedule is owner-major.
        streams = []   # per core: {(owner,bucket): [tile edge lists]}
        for c in range(NCORES):
            sel = np.nonzero(core_of == c)[0]
            order = np.argsort(fatrow[sel], kind="stable")
            e = sel[order]
            fr = fatrow[e]
            bkt = fr // BUCKET
            own = dst[e] // dst_sh
            groups = {}
            n = len(e)
            bounds = [0] + list(np.nonzero((bkt[1:] != bkt[:-1]) |
                                           (own[1:] != own[:-1]))[0] + 1) + [n]
            for gi in range(len(bounds) - 1):
                lo, hi = bounds[gi], bounds[gi + 1]
                if lo == hi:
                    continue
                tiles = []
                cur = []
                i = lo
                while i < hi:
                    j = i
                    while j < hi and dst[e[j]] == dst[e[i]]:
                        j += 1
                    if len(cur) + (j - i) > 128:
                        tiles.append(cur)
                        cur = []
                    cur.extend(e[i:j].tolist())
                    i = j
                if cur:
                    tiles.append(cur)
                groups[(int(own[lo]), int(bkt[lo]))] = tiles
            streams.append(groups)

        # common tile counts per (owner, bucket) group
        keys = sorted({k for s in streams for k in s})
        gtiles = {k: max(len(s.get(k, [])) for s in streams) for k in keys}

        # owner-major static schedule of chunks
        self.sched = []
        tile_base = 0
        M0 = max(sum(_pad_up(gtiles[k], 1) for k in keys if k[0] == o)
                 for o in range(NCORES))
        self.Mpad = _pad_up(max(M0 * 128, 128), 128)
        cbase = {o: 0 for o in range(NCORES)}
        for k in keys:
            o, b = k
            nt = gtiles[k]
            for t0 in range(0, nt, self.CH):
                ct = min(self.CH, nt - t0)
                self.sched.append(dict(bucket=b, owner=o, tiles=tile_base,
                                       n_tiles=ct,
                                       compact=o * self.Mpad + cbase[o] * 128))
                tile_base += ct
                cbase[o] += ct
        self.ET = tile_base
        self.E_pad = self.ET * 128
        assert max(cbase.values()) * 128 <= self.Mpad

        # per-core device arrays
        self.zidx, self.didx, self.mask, self.slot, self.midx = [], [], [], [], []
        block_dsts = []  # per core: per owner: dst id per compact row (-1 pad)
        for c in range(NCORES):
            zi = np.zeros(self.E_pad, np.int64)
            di = np.zeros(self.E_pad, np.int64)
            sl = np.zeros(self.E_pad, np.int64)
            mk = np.zeros(self.E_pad, np.float32)
            ublk = [np.full(self.Mpad, -1, np.int64) for _ in range(NCORES)]
            gseen = {}
            for sc in self.sched:
                k = (sc["owner"], sc["bucket"])
                tiles = streams[c].get(k, [])
                t0 = gseen.get(k, 0)
                gseen[k] = t0 + sc["n_tiles"]
                for ti in range(sc["n_tiles"]):
                    if t0 + ti >= len(tiles):
                        continue
                    te = np.array(tiles[t0 + ti], np.int64)
                    base = (sc["tiles"] + ti) * 128
                    n = len(te)
                    zi[base:base + n] = src[te] - c * src_sh
                    di[base:base + n] = fatrow[te] - sc["bucket"] * BUCKET
                    mk[base:base + n] = 1.0
                    d = dst[te]
                    udst, inv = np.unique(d, return_inverse=True)
                    sl[base:base + n] = inv
                    crow = sc["compact"] - sc["owner"] * self.Mpad + ti * 128
                    ublk[sc["owner"]][crow:crow + len(udst)] = udst
            self.zidx.append(_wrap16(zi))
            self.didx.append(_wrap16(di))
            self.slot.append(_edge_layout(sl))
            self.mask.append(_edge_layout(mk))
            block_dsts.append(ublk)
        self.block_dsts = block_dsts
        for c in range(NCORES):
            mi = np.zeros(NCORES * self.Mpad, np.int64)
            rr = np.arange(NCORES * self.Mpad)
            mi[:] = dst_pad + (rr % DUMP)  # pads: unique dump rows per call window
            for i in range(NCORES):
                ub = block_dsts[i][c]
                v = ub >= 0
                mi[i * self.Mpad:(i + 1) * self.Mpad][v] = ub[v] - c * dst_sh
            self.midx.append(_wrap16(mi))

class Cfg:
    def __init__(self, NW, ND, EWW, EWD):
        self.NW, self.ND = NW, ND
        self.SRC_SH = NW // NCORES
        self.DOC_SH = ND // NCORES
        self.SRC_PAD = _pad_up(self.SRC_SH, 128)
        self.DOC_PAD = _pad_up(self.DOC_SH, 128)


def host_prep(cfg, inputs):
    """All integer index prep + per-core input construction."""
    ww = RelPrep(np.asarray(inputs["ww_src"], np.int64), np.asarray(inputs["ww_dst"], np.int64),
                 cfg.NW, cfg.NW, cfg.SRC_SH, cfg.SRC_SH, cfg.SRC_PAD, cfg.SRC_PAD)
    wd = RelPrep(np.asarray(inputs["wd_src"], np.int64), np.asarray(inputs["wd_dst"], np.int64),
                 cfg.NW, cfg.ND, cfg.SRC_SH, cfg.DOC_SH, cfg.SRC_PAD, cfg.DOC_PAD)

    # transposed, chunked, padded node features per core: [128, KC, N_PAD]
    def shard_T(x, sh, pad):
        fin = x.shape[1]
        kc = fin // 128
        out = []
        for c in range(NCORES):
            xs = np.zeros((pad, fin), np.float32)
            xs[:sh] = x[c * sh:(c + 1) * sh]
            xt = np.ascontiguousarray(xs.T.reshape(kc, 128, pad).transpose(1, 0, 2))
            out.append(xt)
        return out

    xw = shard_T(np.asarray(inputs["x_word"], np.float32), cfg.SRC_SH, cfg.SRC_PAD)
    xd = shard_T(np.asarray(inputs["x_doc"], np.float32), cfg.DOC_SH, cfg.DOC_PAD)

    # parameter constant-folding (host): w_elar[fin, 16] per layer =
    # [W_ww·al_ww | W_ww·ar_ww | W_wd·al_wd | W_wd·ar_wd] head-collapsed,
    # and bsum[128] = sum over heads of bias.
    def welar(W, al, ar):
        W = np.asarray(W, np.float32).reshape(-1, H, F)
        return np.stack([(W * np.asarray(a, np.float32)[None]).sum(-1)  # [fin, H]
                         for a in (al, ar)], axis=-1).reshape(-1, H * 2, order="F")

    # note: build [fin, 8] with cols 0:4 = el heads, 4:8 = er heads
    def welar8(W, al, ar):
        W = np.asarray(W, np.float32).reshape(-1, H, F)
        el = (W * np.asarray(al, np.float32)[None]).sum(-1)
        er = (W * np.asarray(ar, np.float32)[None]).sum(-1)
        return np.concatenate([el, er], axis=1).astype(np.float32)  # [fin, 8]

    params = {}
    for l in range(2):
        Www = np.asarray(inputs[f"W_ww{l}"], np.float32)
        Wwd = np.asarray(inputs[f"W_wd{l}"], np.float32)
        fin = Www.shape[0]
        kc = fin // 128
        params[f"W_ww{l}"] = np.ascontiguousarray(Www.reshape(kc, 128, HF).transpose(1, 0, 2))
        params[f"W_wd{l}"] = np.ascontiguousarray(Wwd.reshape(kc, 128, HF).transpose(1, 0, 2))
        ea = np.concatenate([welar8(Www, inputs[f"al_ww{l}"], inputs[f"ar_ww{l}"]),
                             welar8(Wwd, inputs[f"al_wd{l}"], inputs[f"ar_wd{l}"])],
                            axis=1)  # [fin, 16]
        params[f"elar{l}"] = np.ascontiguousarray(ea.reshape(kc, 128, 16).transpose(1, 0, 2))
        params[f"bsum_ww{l}"] = np.asarray(inputs[f"b_ww{l}"], np.float32).reshape(H, F).sum(0)
        params[f"bsum_wd{l}"] = np.asarray(inputs[f"b_wd{l}"], np.float32).reshape(H, F).sum(0)

    return ww, wd, xw, xd, params


# ---------------------------------------------------------------------------
# device program
# ---------------------------------------------------------------------------

def build_nc(cfg, ww, wd, mm_dt_name="float32"):
    import concourse.bass as bass
    import concourse.mybir as mybir
    import concourse.tile as tile
    from contextlib import ExitStack

    f32 = mybir.dt.float32
    i16 = mybir.dt.int16
    mm_dt = getattr(mybir.dt, mm_dt_name)
    AluOp = mybir.AluOpType
    Act = mybir.ActivationFunctionType

    import concourse.bacc as bacc
    nc = bacc.Bacc(None, target_bir_lowering=False, num_devices=NCORES)
    RG = [list(range(NCORES))]

    SP, DP = cfg.SRC_PAD, cfg.DOC_PAD
    WT, DT_ = SP // 128, DP // 128  # node tiles per core

    # ---------------- external I/O ----------------
    ext_in = {}

    def ein(name, shape, dtype=f32):
        t = nc.dram_tensor(name, list(shape), dtype, kind="ExternalInput")
        ext_in[name] = shape
        return t

    ident_in = ein("ident", (128, 128))
    xw0 = ein("xw0", (128, 2, SP))
    xd0 = ein("xd0", (128, 2, DP))
    Wt = {f"W_{r}{l}": ein(f"W_{r}{l}", (128, (2 if l == 0 else 1), HF))
          for l in range(2) for r in ("ww", "wd")}
    elar_t = {l: ein(f"elar{l}", (128, (2 if l == 0 else 1), 16)) for l in range(2)}
    bsum_t = {(r, l): ein(f"bsum_{r}{l}", (1, F)) for l in range(2) for r in ("ww", "wd")}

    rels = {"ww": ww, "wd": wd}
    idx_t, mask_t, slot_t = {}, {}, {}
    for r, rp in rels.items():
        for nm in ("zidx", "didx"):
            idx_t[(r, nm)] = ein(f"{r}_{nm}", (128, rp.E_pad // 16), i16)
        idx_t[(r, "midx")] = ein(f"{r}_midx", (128, NCORES * rp.Mpad // 16), i16)
        mask_t[r] = ein(f"{r}_mask", (128, rp.ET))
        slot_t[r] = ein(f"{r}_slot", (128, rp.ET))
    iota_t = ein("iota_row", (128, 128))

    yw = nc.dram_tensor("yw", [SP, F], f32, kind="ExternalOutput")
    yd = nc.dram_tensor("yd", [DP, F], f32, kind="ExternalOutput")

    # ---------------- internal DRAM ----------------
    def idram(name, shape, shared=False):
        return nc.dram_tensor(name, list(shape), f32,
                              addr_space=("Shared" if shared else "Local"))

    z_tab = {"ww": idram("z_ww", (SP, HF)), "wd": idram("z_wd", (SP, HF))}
    elfat = idram("elfat", (SP, FATW))
    er_loc = {"ww": idram("er_ww_loc", (SP, FATW)), "wd": idram("er_wd_loc", (DP, FATW))}
    er_fat = {"ww": idram("er_ww_fat", (NCORES * SP, FATW), True),
              "wd": idram("er_wd_fat", (NCORES * DP, FATW), True)}
    invs_loc = {"ww": idram("invs_ww_loc", (SP, FATW)), "wd": idram("invs_wd_loc", (DP, FATW))}
    invs_fat = {"ww": idram("invs_ww_fat", (NCORES * SP, FATW), True),
                "wd": idram("invs_wd_fat", (NCORES * DP, FATW), True)}
    s_send, s_recv, m_send, m_recv, sown, oacc = {}, {}, {}, {}, {}, {}
    for l in range(2):
        for r, rp in rels.items():
            npd = SP if r == "ww" else DP
            s_send[(r, l)] = idram(f"s_send_{r}{l}", (NCORES * rp.Mpad, FATW))
            s_recv[(r, l)] = idram(f"s_recv_{r}{l}", (NCORES * rp.Mpad, FATW))
            m_send[(r, l)] = idram(f"m_send_{r}{l}", (NCORES * rp.Mpad, F))
            m_recv[(r, l)] = idram(f"m_recv_{r}{l}", (NCORES * rp.Mpad, F))
            sown[(r, l)] = idram(f"sown_{r}{l}", (npd + 2048, FATW))
            oacc[(r, l)] = idram(f"oacc_{r}{l}", (npd + 2048, F))

    # ---------------- program ----------------
    PASS1_CH = 32   # tiles (4096 edges) per pass-1 chunk
    PASS2_CH = 8    # tiles (1024 edges) per pass-2 chunk

    with tile.TileContext(nc) as tc, ExitStack() as ctx:
        const = ctx.enter_context(tc.tile_pool(name="const", bufs=1))
        sb = ctx.enter_context(tc.tile_pool(name="sb", bufs=3))
        sb2 = ctx.enter_context(tc.tile_pool(name="sb2", bufs=2))
        ps = ctx.enter_context(tc.tile_pool(name="ps", bufs=2, space="PSUM"))
        pst = ctx.enter_context(tc.tile_pool(name="pst", bufs=1, space="PSUM"))

        _regs = {}

        def nreg(v):
            if v not in _regs:
                _regs[v] = nc.gpsimd.to_reg(v)
            return _regs[v]

        ident = const.tile([128, 128], f32)
        nc.sync.dma_start(out=ident[:], in_=ident_in[:])

        zeros = const.tile([128, 1024], f32, tag="zeros")
        nc.vector.memset(zeros[:], 0.0)

        # -------- zero all DRAM accumulators (overlaps dense L0) --------
        def zero_dram(t):
            n = 1
            for s in t.shape:
                n *= s
            assert n % 128 == 0
            v = t[:].rearrange("a b -> (a b)").rearrange("(p n) -> p n", p=128)
            step = 1024
            for o in range(0, v.shape[1], step):
                w = min(step, v.shape[1] - o)
                nc.sync.dma_start(out=v[:, o:o + w], in_=zeros[:, :w])

        for l in range(2):
            for r in rels:
                zero_dram(s_send[(r, l)])
                zero_dram(m_send[(r, l)])
                zero_dram(sown[(r, l)])
                zero_dram(oacc[(r, l)])

        # -------- persistent SBUF: indices, masks, weights --------
        loaded = {}
        for (key, t) in idx_t.items():
            tl = const.tile([128, t.shape[1]], i16, tag=f"idx_{key[0]}_{key[1]}")
            nc.sync.dma_start(out=tl[:], in_=t[:])
            loaded[key] = tl
        for r, t in mask_t.items():
            tl = const.tile([128, t.shape[1]], f32, tag=f"mask_{r}")
            nc.sync.dma_start(out=tl[:], in_=t[:])
            loaded[("mask", r)] = tl
        for r, t in slot_t.items():
            tl = const.tile([128, t.shape[1]], f32, tag=f"slot_{r}")
            nc.sync.dma_start(out=tl[:], in_=t[:])
            loaded[("slot", r)] = tl
        iota_row = const.tile([128, 128], f32)
        nc.sync.dma_start(out=iota_row[:], in_=iota_t[:])

        W_sb, elar_sb, bsum_sb = {}, {}, {}
        for l in range(2):
            kc = 2 if l == 0 else 1
            for r in ("ww", "wd"):
                tl = const.tile([128, kc, HF], f32, tag=f"W_{r}{l}")
                nc.sync.dma_start(out=tl[:], in_=Wt[f"W_{r}{l}"][:])
                W_sb[(r, l)] = tl
            tl = const.tile([128, kc, 16], f32, tag=f"elar{l}")
            nc.sync.dma_start(out=tl[:], in_=elar_t[l][:])
            elar_sb[l] = tl
            for r in ("ww", "wd"):
                b1 = const.tile([1, F], f32, tag=f"bs1_{r}{l}")
                nc.sync.dma_start(out=b1[:], in_=bsum_t[(r, l)][:])
                bN = const.tile([128, F], f32, tag=f"bsN_{r}{l}")
                nc.gpsimd.partition_broadcast(bN[:], b1[:])
                bsum_sb[(r, l)] = bN

        if mm_dt != f32:
            def mmcast(ap):
                return ap.bitcast(mm_dt)
        else:
            def mmcast(ap):
                return ap

        # ================= dense phase =================
        def dense(l):
            kc = 2 if l == 0 else 1
            for kind, ntile, n_pad in (("w", WT, SP), ("d", DT_, DP)):
                for t in range(ntile):
                    # ---- obtain lhsT chunks [128, kc*128] ----
                    lhsT = sb.tile([128, kc, 128], f32, tag="lhsT")
                    if l == 0:
                        src = xw0 if kind == "w" else xd0
                        nc.sync.dma_start(out=lhsT[:], in_=src[:, :, t * 128:(t + 1) * 128])
                    else:
                        r0 = "ww" if kind == "w" else "wd"
                        rows = sb.tile([128, F], f32, tag="rows_in")
                        nc.sync.dma_start(out=rows[:],
                                          in_=oacc[(r0, 0)][t * 128:(t + 1) * 128, :])
                        # x = relu(acc + bsum)
                        nc.vector.tensor_tensor(out=rows[:], in0=rows[:],
                                                in1=bsum_sb[(r0, 0)][:], op=AluOp.add)
                        nc.scalar.activation(out=rows[:], in_=rows[:], func=Act.Relu)
                        tp = pst.tile([128, 128], f32, space="PSUM", tag="tpose")
                        nc.tensor.transpose(out=tp[:], in_=rows[:], identity=ident[:])
                        nc.vector.tensor_copy(out=lhsT[:, 0, :], in_=tp[:])

                    # ---- matmuls ----
                    pz1 = ps.tile([128, HF], f32, space="PSUM", tag="pz1")
                    pz2 = ps.tile([128, HF], f32, space="PSUM", tag="pz2")
                    pe = pst.tile([128, 16], f32, space="PSUM", tag="pel")
                    for k in range(kc):
                        lk = mmcast(lhsT[:, k, :])
                        st, sp = (k == 0), (k == kc - 1)
                        nc.tensor.matmul(out=pz1[:], lhsT=lk,
                                         rhs=mmcast(W_sb[("ww" if kind == "w" else "wd", l)][:, k, :]),
                                         start=st, stop=sp)
                        if kind == "w":
                            nc.tensor.matmul(out=pz2[:], lhsT=lk,
                                             rhs=mmcast(W_sb[("wd", l)][:, k, :]),
                                             start=st, stop=sp)
                        nc.tensor.matmul(out=pe[:], lhsT=lk, rhs=mmcast(elar_sb[l][:, k, :]),
                                         start=st, stop=sp)

                    if kind == "w":
                        # z tables
                        zs1 = sb.tile([128, HF], f32, tag="zs1")
                        nc.any.tensor_copy(out=zs1[:], in_=pz1[:])
                        nc.sync.dma_start(out=z_tab["ww"][t * 128:(t + 1) * 128, :], in_=zs1[:])
                        zs2 = sb.tile([128, HF], f32, tag="zs2")
                        nc.any.tensor_copy(out=zs2[:], in_=pz2[:])
                        nc.sync.dma_start(out=z_tab["wd"][t * 128:(t + 1) * 128, :], in_=zs2[:])
                        # el fat rows: [el_ww(0:4) | el_wd(4:8)]
                        elf = sb.tile([128, FATW], f32, tag="elf")
                        nc.vector.memset(elf[:], 0.0)
                        nc.vector.tensor_copy(out=elf[:, 0:4], in_=pe[:, 0:4])
                        nc.vector.tensor_copy(out=elf[:, 4:8], in_=pe[:, 8:12])
                        nc.sync.dma_start(out=elfat[t * 128:(t + 1) * 128, :], in_=elf[:])
                        # er_ww fat rows
                        erf = sb.tile([128, FATW], f32, tag="erf")
                        nc.vector.memset(erf[:], 0.0)
                        nc.vector.tensor_copy(out=erf[:, 0:4], in_=pe[:, 4:8])
                        nc.sync.dma_start(out=er_loc["ww"][t * 128:(t + 1) * 128, :], in_=erf[:])
                    else:
                        erf = sb.tile([128, FATW], f32, tag="erf")
                        nc.vector.memset(erf[:], 0.0)
                        nc.vector.tensor_copy(out=erf[:, 0:4], in_=pe[:, 12:16])
                        nc.sync.dma_start(out=er_loc["wd"][t * 128:(t + 1) * 128, :], in_=erf[:])

            tc.strict_bb_all_engine_barrier()
            for r in ("ww", "wd"):
                nc.gpsimd.collective_compute(
                    "AllGather", AluOp.bypass, replica_groups=RG,
                    ins=[er_loc[r][:]], outs=[er_fat[r][:]])
            tc.strict_bb_all_engine_barrier()

        # ================= edge phase =================
        def edge_phase(r, l):
            rp = rels[r]
            zt = z_tab[r]
            el_lo, el_hi = (0, 4) if r == "ww" else (4, 8)
            ex_all = const.tile([128, rp.ET, 4], f32, tag=f"ex_{r}")
            fatN = er_fat[r].shape[0]

            def build_S(t_glob):
                S = sb2.tile([128, 128], f32, tag="S_onehot")
                nc.vector.tensor_scalar(
                    out=S[:], in0=iota_row[:],
                    scalar1=loaded[("slot", r)][:, t_glob:t_glob + 1], scalar2=None,
                    op0=AluOp.is_equal)
                return S

            # ---- pass 1: ex; merged per-tile s rows -> plain writes ----
            for sc in rp.sched:
                t0, cn = sc["tiles"], sc["n_tiles"]
                b = sc["bucket"]
                ne = cn * 128
                elg = sb2.tile([128, PASS2_CH, FATW], f32, tag="elg")
                erg = sb2.tile([128, PASS2_CH, FATW], f32, tag="erg")
                nc.gpsimd.dma_gather(
                    out_ap=elg[:, :cn, :], in_ap=elfat[:],
                    idxs_ap=loaded[(r, "zidx")][:, t0 * 8:(t0 + cn) * 8],
                    num_idxs=ne, num_idxs_reg=nreg(ne), elem_size=FATW)
                nc.gpsimd.dma_gather(
                    out_ap=erg[:, :cn, :],
                    in_ap=er_fat[r][b * BUCKET:min((b + 1) * BUCKET, fatN), :],
                    idxs_ap=loaded[(r, "didx")][:, t0 * 8:(t0 + cn) * 8],
                    num_idxs=ne, num_idxs_reg=nreg(ne), elem_size=FATW)
                ex = ex_all[:, t0:t0 + cn, :]
                nc.vector.tensor_tensor(out=ex, in0=elg[:, :cn, el_lo:el_hi],
                                        in1=erg[:, :cn, 0:4], op=AluOp.add)
                nc.vector.scalar_tensor_tensor(out=ex, in0=ex, scalar=0.2,
                                               in1=ex, op0=AluOp.mult, op1=AluOp.max)
                nc.scalar.activation(out=ex, in_=ex, func=Act.Exp)
                mk = loaded[("mask", r)][:, t0:t0 + cn]
                nc.vector.tensor_tensor(out=ex, in0=ex,
                                        in1=mk.unsqueeze(2).to_broadcast([128, cn, 4]),
                                        op=AluOp.mult)
                # per-tile one-hot merge of ex -> staging fat rows
                stg = sb2.tile([128, PASS2_CH, FATW], f32, tag="sstg")
                nc.vector.memset(stg[:], 0.0)
                for ti in range(cn):
                    S = build_S(t0 + ti)
                    pm = pst.tile([128, F], f32, space="PSUM", tag="pmg", bufs=2)
                    nc.tensor.matmul(out=pm[:, 0:4], lhsT=S[:],
                                     rhs=ex_all[:, t0 + ti, :], start=True, stop=True)
                    nc.vector.tensor_copy(out=stg[:, ti, 0:4], in_=pm[:, 0:4])
                nc.sync.dma_start(
                    out=s_send[(r, l)][:].rearrange("(t p) c -> p t c", p=128)[
                        :, sc["compact"] // 128:sc["compact"] // 128 + cn, :],
                    in_=stg[:, :cn, :])

            tc.strict_bb_all_engine_barrier()
            nc.gpsimd.collective_compute("AllToAll", AluOp.bypass, replica_groups=RG,
                                         ins=[s_send[(r, l)][:]], outs=[s_recv[(r, l)][:]])
            tc.strict_bb_all_engine_barrier()

            # ---- owner: merge s (one scatter_add per sender block) ----
            MT = rp.Mpad // 128
            for i in range(NCORES):
                tc.strict_bb_all_engine_barrier()
                for c0 in range(0, MT, PASS1_CH):
                    cn = min(PASS1_CH, MT - c0)
                    rin = sb2.tile([128, PASS1_CH, FATW], f32, tag="srecv")
                    nc.sync.dma_start(
                        out=rin[:, :cn, :],
                        in_=s_recv[(r, l)][:].rearrange("(t p) c -> p t c", p=128)[
                            :, i * MT + c0:i * MT + c0 + cn, :])
                    nc.gpsimd.dma_scatter_add(
                        out_ap=sown[(r, l)][:], in_ap=rin[:, :cn, :],
                        idxs_ap=loaded[(r, "midx")][:, (i * MT + c0) * 8:(i * MT + c0 + cn) * 8],
                        num_idxs=cn * 128, num_idxs_reg=nreg(cn * 128),
                        elem_size=FATW)

            npd = SP if r == "ww" else DP
            nt = npd // 128
            sfa = sown[(r, l)][:].rearrange("(t p) c -> p t c", p=128)
            iva = invs_loc[r][:].rearrange("(t p) c -> p t c", p=128)
            for n0 in range(0, nt, 16):
                nn = min(16, nt - n0)
                stile = sb.tile([128, 16, FATW], f32, tag="stile")
                nc.sync.dma_start(out=stile[:, :nn, :], in_=sfa[:, n0:n0 + nn, :])
                sv = stile[:, :nn, 0:4]
                nc.vector.tensor_scalar(out=sv, in0=sv, scalar1=1e-30, scalar2=None,
                                        op0=AluOp.max)
                nc.vector.reciprocal(out=sv, in_=sv)
                nc.sync.dma_start(out=iva[:, n0:n0 + nn, :], in_=stile[:, :nn, :])
            tc.strict_bb_all_engine_barrier()
            nc.gpsimd.collective_compute("AllGather", AluOp.bypass, replica_groups=RG,
                                         ins=[invs_loc[r][:]], outs=[invs_fat[r][:]])
            tc.strict_bb_all_engine_barrier()

            # ---- pass 2: alpha-weighted messages, per-tile merge, plain write ----
            ivN = invs_fat[r].shape[0]
            for sc in rp.sched:
                t0, cn = sc["tiles"], sc["n_tiles"]
                b = sc["bucket"]
                ne = cn * 128
                ivg = sb2.tile([128, PASS2_CH, FATW], f32, tag="ivg")
                nc.gpsimd.dma_gather(
                    out_ap=ivg[:, :cn, :],
                    in_ap=invs_fat[r][b * BUCKET:min((b + 1) * BUCKET, ivN), :],
                    idxs_ap=loaded[(r, "didx")][:, t0 * 8:(t0 + cn) * 8],
                    num_idxs=ne, num_idxs_reg=nreg(ne), elem_size=FATW)
                alp = sb2.tile([128, PASS2_CH, 4], f32, tag="alp")
                nc.vector.tensor_tensor(out=alp[:, :cn, :], in0=ex_all[:, t0:t0 + cn, :],
                                        in1=ivg[:, :cn, 0:4], op=AluOp.mult)
                zg = sb2.tile([128, PASS2_CH, HF], f32, tag="zg")
                nc.gpsimd.dma_gather(
                    out_ap=zg[:, :cn, :], in_ap=zt[:],
                    idxs_ap=loaded[(r, "zidx")][:, t0 * 8:(t0 + cn) * 8],
                    num_idxs=ne, num_idxs_reg=nreg(ne), elem_size=HF)
                mst = sb2.tile([128, PASS2_CH, F], f32, tag="mstg")
                for ti in range(cn):
                    msg = sb2.tile([128, F], f32, tag="msg")
                    for h in range(H):
                        zh = zg[:, ti, h * F:(h + 1) * F]
                        a = alp[:, ti, h:h + 1]
                        if h == 0:
                            nc.vector.tensor_scalar(out=msg[:], in0=zh, scalar1=a,
                                                    scalar2=None, op0=AluOp.mult)
                        else:
                            nc.vector.scalar_tensor_tensor(
                                out=msg[:], in0=zh, scalar=a, in1=msg[:],
                                op0=AluOp.mult, op1=AluOp.add)
                    S = build_S(t0 + ti)
                    pm = pst.tile([128, F], f32, space="PSUM", tag="pmg", bufs=2)
                    nc.tensor.matmul(out=pm[:], lhsT=S[:], rhs=msg[:],
                                     start=True, stop=True)
                    nc.vector.tensor_copy(out=mst[:, ti, :], in_=pm[:])
                nc.sync.dma_start(
                    out=m_send[(r, l)][:].rearrange("(t p) c -> p t c", p=128)[
                        :, sc["compact"] // 128:sc["compact"] // 128 + cn, :],
                    in_=mst[:, :cn, :])

            tc.strict_bb_all_engine_barrier()
            nc.gpsimd.collective_compute("AllToAll", AluOp.bypass, replica_groups=RG,
                                         ins=[m_send[(r, l)][:]], outs=[m_recv[(r, l)][:]])
            tc.strict_bb_all_engine_barrier()

            # ---- owner merge messages (one scatter_add per sender block) ----
            for i in range(NCORES):
                tc.strict_bb_all_engine_barrier()
                for c0 in range(0, MT, PASS2_CH):
                    cn = min(PASS2_CH, MT - c0)
                    rin = sb2.tile([128, PASS2_CH, F], f32, tag="mrecv")
                    nc.sync.dma_start(
                        out=rin[:, :cn, :],
                        in_=m_recv[(r, l)][:].rearrange("(t p) c -> p t c", p=128)[
                            :, i * MT + c0:i * MT + c0 + cn, :])
                    nc.gpsimd.dma_scatter_add(
                        out_ap=oacc[(r, l)][:], in_ap=rin[:, :cn, :],
                        idxs_ap=loaded[(r, "midx")][:, (i * MT + c0) * 8:(i * MT + c0 + cn) * 8],
                        num_idxs=cn * 128, num_idxs_reg=nreg(cn * 128), elem_size=F)

        # ================= full schedule =================
        dense(0)
        edge_phase("ww", 0)
        edge_phase("wd", 0)
        import os as _os
        if _os.environ.get("GAT_DEBUG"):
            dbg = {"d_z": z_tab["ww"], "d_elfat": elfat, "d_erfat": er_fat["ww"],
                   "d_ssend": s_send[("ww", 0)], "d_srecv": s_recv[("ww", 0)],
                   "d_sown": sown[("ww", 0)], "d_invs": invs_fat["ww"],
                   "d_msend": m_send[("ww", 0)], "d_oacc": oacc[("ww", 0)]}
            tc.strict_bb_all_engine_barrier()
            for nm, t in dbg.items():
                o = nc.dram_tensor(nm, list(t.shape), f32, kind="ExternalOutput")
                v = t[:].rearrange("a b -> (a b)").rearrange("(p n) -> p n", p=128)
                vo = o[:].rearrange("a b -> (a b)").rearrange("(p n) -> p n", p=128)
                for c0 in range(0, v.shape[1], 512):
                    w = min(512, v.shape[1] - c0)
                    tmp = sb.tile([128, 512], f32, tag="dbgcp")
                    nc.sync.dma_start(out=tmp[:, :w], in_=v[:, c0:c0 + w])
                    nc.sync.dma_start(out=vo[:, c0:c0 + w], in_=tmp[:, :w])
        dense(1)
        edge_phase("ww", 1)
        edge_phase("wd", 1)

        # final: y = relu(acc + bsum)
        for (r, out_t, nt) in (("ww", yw, WT), ("wd", yd, DT_)):
            for t in range(nt):
                rows = sb.tile([128, F], f32, tag="rows_out")
                nc.sync.dma_start(out=rows[:], in_=oacc[(r, 1)][t * 128:(t + 1) * 128, :])
                nc.vector.tensor_tensor(out=rows[:], in0=rows[:],
                                        in1=bsum_sb[(r, 1)][:], op=AluOp.add)
                nc.scalar.activation(out=rows[:], in_=rows[:], func=Act.Relu)
                nc.sync.dma_start(out=out_t[t * 128:(t + 1) * 128, :], in_=rows[:])

    nc.compile()
    return nc


# ---------------------------------------------------------------------------
# entry point
# ---------------------------------------------------------------------------

def _make_in_maps(cfg, ww, wd, xw, xd, params):
    in_maps = []
    for c in range(NCORES):
        m = {
            "ident": np.eye(128, dtype=np.float32),
            "xw0": xw[c], "xd0": xd[c],
            "W_ww0": params["W_ww0"], "W_wd0": params["W_wd0"],
            "W_ww1": params["W_ww1"], "W_wd1": params["W_wd1"],
            "elar0": params["elar0"], "elar1": params["elar1"],
        }
        for l in range(2):
            for r in ("ww", "wd"):
                m[f"bsum_{r}{l}"] = params[f"bsum_{r}{l}"].reshape(1, F)
        m["iota_row"] = np.broadcast_to(np.arange(128, dtype=np.float32), (128, 128)).copy()
        for r, rp in (("ww", ww), ("wd", wd)):
            m[f"{r}_zidx"] = rp.zidx[c]
            m[f"{r}_didx"] = rp.didx[c]
            m[f"{r}_slot"] = rp.slot[c]
            m[f"{r}_midx"] = rp.midx[c]
            m[f"{r}_mask"] = rp.mask[c]
        in_maps.append(m)
    return in_maps


def run(inputs, cfg=None, trace=False, mm_dt="float32"):
    from concourse.bass_utils import run_bass_kernel_spmd
    if cfg is None:
        cfg = Cfg(80000, 16000, 200000, 200000)
    ww, wd, xw, xd, params = host_prep(cfg, inputs)
    nc = build_nc(cfg, ww, wd, mm_dt_name=mm_dt)
    in_maps = _make_in_maps(cfg, ww, wd, xw, xd, params)
    res = run_bass_kernel_spmd(nc, in_maps, list(range(NCORES)), trace=trace)
    outw = np.concatenate([res.results[c]["yw"][:cfg.SRC_SH] for c in range(NCORES)], 0)
    outd = np.concatenate([res.results[c]["yd"][:cfg.DOC_SH] for c in range(NCORES)], 0)
    return (outw, outd), res


def kernel(**inputs):
    (outw, outd), _ = run(inputs)
    return outw, outd


# revision 23
# speedup vs baseline: 1.0202x; 1.0202x over previous
"""2-layer heterogeneous GAT (word-word + word-doc relations) on 8 TRN2 cores.

Strategy (edge-parallel, src-sharded):
  - Words/docs row-sharded across 8 cores. Edges assigned to core(src//SH),
    sorted by destination. Each core computes z = x@W for its node rows only.
  - Per-edge attention logits via "fat-row" (256B) dma_gather from local
    el-tables and AllGathered er-tables.
  - Softmax denominators s: per-edge ex scattered (dma_scatter_add) into
    compact per-destination-owner rows, AllToAll'd to owners, merged,
    reciprocal'd, AllGathered back as a fat inv_s table.
  - Messages: gather z rows (2KB) by src (local table), weight by
    alpha = ex*inv_s on DVE with per-head fused mul-adds (head sum folded in),
    scatter-add into compact per-owner rows, AllToAll, owner merges into its
    dense output shard. Bias+relu fused into next layer's load.
  - Outputs are per-core shards; host concatenates. No full-size AllReduduce.
"""

import numpy as np

P = 128
NCORES = 8
H = 4
F = 128
HF = 512
FATW = 64          # fat-row width (f32 words) for 256B-row gather tables
BUCKET = 32768     # int16 index range per dma_gather bucket
DUMP = 2048        # dump rows appended to owner shards for scatter pads


def _pad_up(x, m):
    return (x + m - 1) // m * m


def _wrap16(idx, dtype=np.int16):
    """Layout an index vector the way dma_gather/dma_scatter_add expect:
    [16, n/16] with element i at [i % 16, i // 16]."""
    n = len(idx)
    assert n % 16 == 0
    w = np.ascontiguousarray(idx.reshape(n // 16, 16).T).astype(dtype)
    return np.tile(w, (8, 1))  # replicated across the 8 GPSIMD cores


def _edge_layout(arr, dtype=np.float32):
    """Per-edge array -> [128, ntiles] with edge i at [i % 128, i // 128]."""
    n = len(arr)
    assert n % 128 == 0
    return np.ascontiguousarray(arr.reshape(n // 128, 128).T).astype(dtype)


class RelPrep:
    """Host-side index prep for one relation (shared by both layers)."""

    def __init__(self, src, dst, n_src, n_dst, src_sh, dst_sh, src_pad, dst_pad,
                 chunk_tiles=8):
        self.src_sh, self.dst_sh = src_sh, dst_sh
        self.src_pad, self.dst_pad = src_pad, dst_pad
        self.CH = chunk_tiles

        core_of = src // src_sh
        fatrow = (dst // dst_sh) * dst_pad + dst % dst_sh

        # Per-core edge stream sorted by (owner, dst) == fatrow, packed into
        # 128-edge tiles with segment alignment (no dst straddles a tile).
        # Chunks of <=CH tiles never cross bucket or owner boundaries.
        # Compact/send space: one 128-row group per tile (row = tile*128 +
        # tile-local unique-dst slot). Owner blocks are contiguous because the
        # schedule is owner-major.
        streams = []   # per core: {(owner,bucket): [tile edge lists]}
        for c in range(NCORES):
            sel = np.nonzero(core_of == c)[0]
            order = np.argsort(fatrow[sel], kind="stable")
            e = sel[order]
            fr = fatrow[e]
            bkt = fr // BUCKET
            own = dst[e] // dst_sh
            groups = {}
            n = len(e)
            bounds = [0] + list(np.nonzero((bkt[1:] != bkt[:-1]) |
                                           (own[1:] != own[:-1]))[0] + 1) + [n]
            for gi in range(len(bounds) - 1):
                lo, hi = bounds[gi], bounds[gi + 1]
                if lo == hi:
                    continue
                tiles = []
                cur = []
                i = lo
                while i < hi:
                    j = i
                    while j < hi and dst[e[j]] == dst[e[i]]:
                        j += 1
                    if len(cur) + (j - i) > 128:
                        tiles.append(cur)
                        cur = []
                    cur.extend(e[i:j].tolist())
                    i = j
                if cur:
                    tiles.append(cur)
                groups[(int(own[lo]), int(bkt[lo]))] = tiles
            streams.append(groups)

        # common tile counts per (owner, bucket) group
        keys = sorted({k for s in streams for k in s})
        gtiles = {k: max(len(s.get(k, [])) for s in streams) for k in keys}

        # owner-major static schedule of chunks
        self.sched = []
        tile_base = 0
        M0 = max(sum(_pad_up(gtiles[k], 1) for k in keys if k[0] == o)
                 for o in range(NCORES))
        self.Mpad = _pad_up(max(M0 * 128, 128), 128)
        cbase = {o: 0 for o in range(NCORES)}
        for k in keys:
            o, b = k
            nt = gtiles[k]
            for t0 in range(0, nt, self.CH):
                ct = min(self.CH, nt - t0)
                self.sched.append(dict(bucket=b, owner=o, tiles=tile_base,
                                       n_tiles=ct,
                                       compact=o * self.Mpad + cbase[o] * 128))
                tile_base += ct
                cbase[o] += ct
        self.ET = tile_base
        self.E_pad = self.ET * 128
        assert max(cbase.values()) * 128 <= self.Mpad

        # per-core device arrays
        self.zidx, self.didx, self.mask, self.slot, self.midx = [], [], [], [], []
        block_dsts = []  # per core: per owner: dst id per compact row (-1 pad)
        for c in range(NCORES):
            zi = np.zeros(self.E_pad, np.int64)
            di = np.zeros(self.E_pad, np.int64)
            sl = np.zeros(self.E_pad, np.int64)
            mk = np.zeros(self.E_pad, np.float32)
            ublk = [np.full(self.Mpad, -1, np.int64) for _ in range(NCORES)]
            gseen = {}
            for sc in self.sched:
                k = (sc["owner"], sc["bucket"])
                tiles = streams[c].get(k, [])
                t0 = gseen.get(k, 0)
                gseen[k] = t0 + sc["n_tiles"]
                for ti in range(sc["n_tiles"]):
                    if t0 + ti >= len(tiles):
                        continue
                    te = np.array(tiles[t0 + ti], np.int64)
                    base = (sc["tiles"] + ti) * 128
                    n = len(te)
                    zi[base:base + n] = src[te] - c * src_sh
                    di[base:base + n] = fatrow[te] - sc["bucket"] * BUCKET
                    mk[base:base + n] = 1.0
                    d = dst[te]
                    udst, inv = np.unique(d, return_inverse=True)
                    sl[base:base + n] = inv
                    crow = sc["compact"] - sc["owner"] * self.Mpad + ti * 128
                    ublk[sc["owner"]][crow:crow + len(udst)] = udst
            self.zidx.append(_wrap16(zi))
            self.didx.append(_wrap16(di))
            self.slot.append(_edge_layout(sl))
            self.mask.append(_edge_layout(mk))
            block_dsts.append(ublk)
        self.block_dsts = block_dsts
        for c in range(NCORES):
            mi = np.zeros(NCORES * self.Mpad, np.int64)
            rr = np.arange(NCORES * self.Mpad)
            mi[:] = dst_pad + (rr % DUMP)  # pads: unique dump rows per call window
            for i in range(NCORES):
                ub = block_dsts[i][c]
                v = ub >= 0
                mi[i * self.Mpad:(i + 1) * self.Mpad][v] = ub[v] - c * dst_sh
            self.midx.append(_wrap16(mi))

class Cfg:
    def __init__(self, NW, ND, EWW, EWD):
        self.NW, self.ND = NW, ND
        self.SRC_SH = NW // NCORES
        self.DOC_SH = ND // NCORES
        self.SRC_PAD = _pad_up(self.SRC_SH, 128)
        self.DOC_PAD = _pad_up(self.DOC_SH, 128)


def host_prep(cfg, inputs):
    """All integer index prep + per-core input construction."""
    ww = RelPrep(np.asarray(inputs["ww_src"], np.int64), np.asarray(inputs["ww_dst"], np.int64),
                 cfg.NW, cfg.NW, cfg.SRC_SH, cfg.SRC_SH, cfg.SRC_PAD, cfg.SRC_PAD)
    wd = RelPrep(np.asarray(inputs["wd_src"], np.int64), np.asarray(inputs["wd_dst"], np.int64),
                 cfg.NW, cfg.ND, cfg.SRC_SH, cfg.DOC_SH, cfg.SRC_PAD, cfg.DOC_PAD)

    # transposed, chunked, padded node features per core: [128, KC, N_PAD]
    def shard_T(x, sh, pad):
        fin = x.shape[1]
        kc = fin // 128
        out = []
        for c in range(NCORES):
            xs = np.zeros((pad, fin), np.float32)
            xs[:sh] = x[c * sh:(c + 1) * sh]
            xt = np.ascontiguousarray(xs.T.reshape(kc, 128, pad).transpose(1, 0, 2))
            out.append(xt)
        return out

    xw = shard_T(np.asarray(inputs["x_word"], np.float32), cfg.SRC_SH, cfg.SRC_PAD)
    xd = shard_T(np.asarray(inputs["x_doc"], np.float32), cfg.DOC_SH, cfg.DOC_PAD)

    # parameter constant-folding (host): w_elar[fin, 16] per layer =
    # [W_ww·al_ww | W_ww·ar_ww | W_wd·al_wd | W_wd·ar_wd] head-collapsed,
    # and bsum[128] = sum over heads of bias.
    def welar(W, al, ar):
        W = np.asarray(W, np.float32).reshape(-1, H, F)
        return np.stack([(W * np.asarray(a, np.float32)[None]).sum(-1)  # [fin, H]
                         for a in (al, ar)], axis=-1).reshape(-1, H * 2, order="F")

    # note: build [fin, 8] with cols 0:4 = el heads, 4:8 = er heads
    def welar8(W, al, ar):
        W = np.asarray(W, np.float32).reshape(-1, H, F)
        el = (W * np.asarray(al, np.float32)[None]).sum(-1)
        er = (W * np.asarray(ar, np.float32)[None]).sum(-1)
        return np.concatenate([el, er], axis=1).astype(np.float32)  # [fin, 8]

    params = {}
    for l in range(2):
        Www = np.asarray(inputs[f"W_ww{l}"], np.float32)
        Wwd = np.asarray(inputs[f"W_wd{l}"], np.float32)
        fin = Www.shape[0]
        kc = fin // 128
        params[f"W_ww{l}"] = np.ascontiguousarray(Www.reshape(kc, 128, HF).transpose(1, 0, 2))
        params[f"W_wd{l}"] = np.ascontiguousarray(Wwd.reshape(kc, 128, HF).transpose(1, 0, 2))
        ea = np.concatenate([welar8(Www, inputs[f"al_ww{l}"], inputs[f"ar_ww{l}"]),
                             welar8(Wwd, inputs[f"al_wd{l}"], inputs[f"ar_wd{l}"])],
                            axis=1)  # [fin, 16]
        params[f"elar{l}"] = np.ascontiguousarray(ea.reshape(kc, 128, 16).transpose(1, 0, 2))
        params[f"bsum_ww{l}"] = np.asarray(inputs[f"b_ww{l}"], np.float32).reshape(H, F).sum(0)
        params[f"bsum_wd{l}"] = np.asarray(inputs[f"b_wd{l}"], np.float32).reshape(H, F).sum(0)

    return ww, wd, xw, xd, params


# ---------------------------------------------------------------------------
# device program
# ---------------------------------------------------------------------------

def build_nc(cfg, ww, wd, mm_dt_name="float32"):
    import concourse.bass as bass
    import concourse.mybir as mybir
    import concourse.tile as tile
    from contextlib import ExitStack

    f32 = mybir.dt.float32
    i16 = mybir.dt.int16
    mm_dt = getattr(mybir.dt, mm_dt_name)
    AluOp = mybir.AluOpType
    Act = mybir.ActivationFunctionType

    import concourse.bacc as bacc
    nc = bacc.Bacc(None, target_bir_lowering=False, num_devices=NCORES)
    RG = [list(range(NCORES))]

    SP, DP = cfg.SRC_PAD, cfg.DOC_PAD
    WT, DT_ = SP // 128, DP // 128  # node tiles per core

    # ---------------- external I/O ----------------
    ext_in = {}

    def ein(name, shape, dtype=f32):
        t = nc.dram_tensor(name, list(shape), dtype, kind="ExternalInput")
        ext_in[name] = shape
        return t

    ident_in = ein("ident", (128, 128))
    xw0 = ein("xw0", (128, 2, SP))
    xd0 = ein("xd0", (128, 2, DP))
    Wt = {f"W_{r}{l}": ein(f"W_{r}{l}", (128, (2 if l == 0 else 1), HF))
          for l in range(2) for r in ("ww", "wd")}
    elar_t = {l: ein(f"elar{l}", (128, (2 if l == 0 else 1), 16)) for l in range(2)}
    bsum_t = {(r, l): ein(f"bsum_{r}{l}", (1, F)) for l in range(2) for r in ("ww", "wd")}

    rels = {"ww": ww, "wd": wd}
    idx_t, mask_t, slot_t = {}, {}, {}
    for r, rp in rels.items():
        for nm in ("zidx", "didx"):
            idx_t[(r, nm)] = ein(f"{r}_{nm}", (128, rp.E_pad // 16), i16)
        idx_t[(r, "midx")] = ein(f"{r}_midx", (128, NCORES * rp.Mpad // 16), i16)
        mask_t[r] = ein(f"{r}_mask", (128, rp.ET))
        slot_t[r] = ein(f"{r}_slot", (128, rp.ET))
    iota_t = ein("iota_row", (128, 128))

    yw = nc.dram_tensor("yw", [SP, F], f32, kind="ExternalOutput")
    yd = nc.dram_tensor("yd", [DP, F], f32, kind="ExternalOutput")

    # ---------------- internal DRAM ----------------
    def idram(name, shape, shared=False):
        return nc.dram_tensor(name, list(shape), f32,
                              addr_space=("Shared" if shared else "Local"))

    z_tab = {"ww": idram("z_ww", (SP, HF)), "wd": idram("z_wd", (SP, HF))}
    elfat = idram("elfat", (SP, FATW))
    er_loc = {"ww": idram("er_ww_loc", (SP, FATW)), "wd": idram("er_wd_loc", (DP, FATW))}
    er_fat = {"ww": idram("er_ww_fat", (NCORES * SP, FATW), True),
              "wd": idram("er_wd_fat", (NCORES * DP, FATW), True)}
    invs_loc = {"ww": idram("invs_ww_loc", (SP, FATW)), "wd": idram("invs_wd_loc", (DP, FATW))}
    invs_fat = {"ww": idram("invs_ww_fat", (NCORES * SP, FATW), True),
                "wd": idram("invs_wd_fat", (NCORES * DP, FATW), True)}
    s_send, s_recv, m_send, m_recv, sown, oacc = {}, {}, {}, {}, {}, {}
    for l in range(2):
        for r, rp in rels.items():
            npd = SP if r == "ww" else DP
            s_send[(r, l)] = idram(f"s_send_{r}{l}", (NCORES * rp.Mpad, FATW))
            s_recv[(r, l)] = idram(f"s_recv_{r}{l}", (NCORES * rp.Mpad, FATW))
            m_send[(r, l)] = idram(f"m_send_{r}{l}", (NCORES * rp.Mpad, F))
            m_recv[(r, l)] = idram(f"m_recv_{r}{l}", (NCORES * rp.Mpad, F))
            sown[(r, l)] = idram(f"sown_{r}{l}", (npd + 2048, FATW))
            oacc[(r, l)] = idram(f"oacc_{r}{l}", (npd + 2048, F))

    # ---------------- program ----------------
    PASS1_CH = 32   # tiles (4096 edges) per pass-1 chunk
    PASS2_CH = 8    # tiles (1024 edges) per pass-2 chunk

    with tile.TileContext(nc) as tc, ExitStack() as ctx:
        const = ctx.enter_context(tc.tile_pool(name="const", bufs=1))
        sb = ctx.enter_context(tc.tile_pool(name="sb", bufs=3))
        sb2 = ctx.enter_context(tc.tile_pool(name="sb2", bufs=2))
        ps = ctx.enter_context(tc.tile_pool(name="ps", bufs=2, space="PSUM"))
        pst = ctx.enter_context(tc.tile_pool(name="pst", bufs=1, space="PSUM"))

        _regs = {}

        def nreg(v):
            if v not in _regs:
                _regs[v] = nc.gpsimd.to_reg(v)
            return _regs[v]

        ident = const.tile([128, 128], f32)
        nc.sync.dma_start(out=ident[:], in_=ident_in[:])

        zeros = const.tile([128, 1024], f32, tag="zeros")
        nc.vector.memset(zeros[:], 0.0)

        # -------- zero all DRAM accumulators (overlaps dense L0) --------
        def zero_dram(t):
            n = 1
            for s in t.shape:
                n *= s
            assert n % 128 == 0
            v = t[:].rearrange("a b -> (a b)").rearrange("(p n) -> p n", p=128)
            step = 1024
            for o in range(0, v.shape[1], step):
                w = min(step, v.shape[1] - o)
                nc.sync.dma_start(out=v[:, o:o + w], in_=zeros[:, :w])

        for l in range(2):
            for r in rels:
                zero_dram(s_send[(r, l)])
                zero_dram(m_send[(r, l)])
                zero_dram(sown[(r, l)])
                zero_dram(oacc[(r, l)])

        # -------- persistent SBUF: indices, masks, weights --------
        loaded = {}
        for (key, t) in idx_t.items():
            tl = const.tile([128, t.shape[1]], i16, tag=f"idx_{key[0]}_{key[1]}")
            nc.sync.dma_start(out=tl[:], in_=t[:])
            loaded[key] = tl
        for r, t in mask_t.items():
            tl = const.tile([128, t.shape[1]], f32, tag=f"mask_{r}")
            nc.sync.dma_start(out=tl[:], in_=t[:])
            loaded[("mask", r)] = tl
        for r, t in slot_t.items():
            tl = const.tile([128, t.shape[1]], f32, tag=f"slot_{r}")
            nc.sync.dma_start(out=tl[:], in_=t[:])
            loaded[("slot", r)] = tl
        iota_row = const.tile([128, 128], f32)
        nc.sync.dma_start(out=iota_row[:], in_=iota_t[:])

        W_sb, elar_sb, bsum_sb = {}, {}, {}
        for l in range(2):
            kc = 2 if l == 0 else 1
            for r in ("ww", "wd"):
                tl = const.tile([128, kc, HF], mm_dt, tag=f"W_{r}{l}")
                nc.sync.dma_start(out=tl[:], in_=Wt[f"W_{r}{l}"][:].bitcast(mm_dt))
                W_sb[(r, l)] = tl
            tl = const.tile([128, kc, 16], mm_dt, tag=f"elar{l}")
            nc.sync.dma_start(out=tl[:], in_=elar_t[l][:].bitcast(mm_dt))
            elar_sb[l] = tl
            for r in ("ww", "wd"):
                b1 = const.tile([1, F], f32, tag=f"bs1_{r}{l}")
                nc.sync.dma_start(out=b1[:], in_=bsum_t[(r, l)][:])
                bN = const.tile([128, F], f32, tag=f"bsN_{r}{l}")
                nc.gpsimd.partition_broadcast(bN[:], b1[:])
                bsum_sb[(r, l)] = bN

        if mm_dt != f32:
            def mmcast(ap):
                return ap.bitcast(mm_dt)
        else:
            def mmcast(ap):
                return ap

        # ================= dense phase =================
        def dense(l):
            kc = 2 if l == 0 else 1
            for kind, ntile, n_pad in (("w", WT, SP), ("d", DT_, DP)):
                for t in range(ntile):
                    # ---- obtain lhsT chunks [128, kc*128] ----
                    lhsT = sb.tile([128, kc, 128], mm_dt, tag="lhsT")
                    if l == 0:
                        src = xw0 if kind == "w" else xd0
                        nc.sync.dma_start(out=lhsT[:],
                                          in_=src[:, :, t * 128:(t + 1) * 128].bitcast(mm_dt))
                    else:
                        r0 = "ww" if kind == "w" else "wd"
                        rows = sb.tile([128, F], f32, tag="rows_in")
                        nc.sync.dma_start(out=rows[:],
                                          in_=oacc[(r0, 0)][t * 128:(t + 1) * 128, :])
                        # x = relu(acc + bsum)
                        nc.vector.tensor_tensor(out=rows[:], in0=rows[:],
                                                in1=bsum_sb[(r0, 0)][:], op=AluOp.add)
                        nc.scalar.activation(out=rows[:], in_=rows[:], func=Act.Relu)
                        tp = pst.tile([128, 128], f32, space="PSUM", tag="tpose")
                        nc.tensor.transpose(out=tp[:], in_=rows[:], identity=ident[:])
                        nc.vector.tensor_copy(out=lhsT[:, 0, :], in_=tp[:])

                    # ---- matmuls ----
                    pz1 = ps.tile([128, HF], f32, space="PSUM", tag="pz1")
                    pz2 = ps.tile([128, HF], f32, space="PSUM", tag="pz2")
                    pe = pst.tile([128, 16], f32, space="PSUM", tag="pel")
                    for k in range(kc):
                        lk = lhsT[:, k, :]
                        st, sp = (k == 0), (k == kc - 1)
                        nc.tensor.matmul(out=pz1[:], lhsT=lk,
                                         rhs=W_sb[("ww" if kind == "w" else "wd", l)][:, k, :],
                                         start=st, stop=sp)
                        if kind == "w":
                            nc.tensor.matmul(out=pz2[:], lhsT=lk,
                                             rhs=W_sb[("wd", l)][:, k, :],
                                             start=st, stop=sp)
                        nc.tensor.matmul(out=pe[:], lhsT=lk, rhs=elar_sb[l][:, k, :],
                                         start=st, stop=sp)

                    if kind == "w":
                        # z tables
                        zs1 = sb.tile([128, HF], f32, tag="zs1")
                        nc.vector.tensor_copy(out=zs1[:], in_=pz1[:])
                        nc.sync.dma_start(out=z_tab["ww"][t * 128:(t + 1) * 128, :], in_=zs1[:])
                        zs2 = sb.tile([128, HF], f32, tag="zs2")
                        nc.vector.tensor_copy(out=zs2[:], in_=pz2[:])
                        nc.sync.dma_start(out=z_tab["wd"][t * 128:(t + 1) * 128, :], in_=zs2[:])
                        # el fat rows: [el_ww(0:4) | el_wd(4:8)]
                        elf = sb.tile([128, FATW], f32, tag="elf")
                        nc.vector.memset(elf[:], 0.0)
                        nc.vector.tensor_copy(out=elf[:, 0:4], in_=pe[:, 0:4])
                        nc.vector.tensor_copy(out=elf[:, 4:8], in_=pe[:, 8:12])
                        nc.sync.dma_start(out=elfat[t * 128:(t + 1) * 128, :], in_=elf[:])
                        # er_ww fat rows
                        erf = sb.tile([128, FATW], f32, tag="erf")
                        nc.vector.memset(erf[:], 0.0)
                        nc.vector.tensor_copy(out=erf[:, 0:4], in_=pe[:, 4:8])
                        nc.sync.dma_start(out=er_loc["ww"][t * 128:(t + 1) * 128, :], in_=erf[:])
                    else:
                        erf = sb.tile([128, FATW], f32, tag="erf")
                        nc.vector.memset(erf[:], 0.0)
                        nc.vector.tensor_copy(out=erf[:, 0:4], in_=pe[:, 12:16])
                        nc.sync.dma_start(out=er_loc["wd"][t * 128:(t + 1) * 128, :], in_=erf[:])

            tc.strict_bb_all_engine_barrier()
            for r in ("ww", "wd"):
                nc.gpsimd.collective_compute(
                    "AllGather", AluOp.bypass, replica_groups=RG,
                    ins=[er_loc[r][:]], outs=[er_fat[r][:]])
            tc.strict_bb_all_engine_barrier()

        # ================= edge phase =================
        def edge_phase(r, l):
            rp = rels[r]
            zt = z_tab[r]
            el_lo, el_hi = (0, 4) if r == "ww" else (4, 8)
            ex_all = const.tile([128, rp.ET, 4], f32, tag=f"ex_{r}")
            fatN = er_fat[r].shape[0]

            def build_S(t_glob):
                S = sb2.tile([128, 128], f32, tag="S_onehot")
                nc.vector.tensor_scalar(
                    out=S[:], in0=iota_row[:],
                    scalar1=loaded[("slot", r)][:, t_glob:t_glob + 1], scalar2=None,
                    op0=AluOp.is_equal)
                return S

            # ---- pass 1: ex; merged per-tile s rows -> plain writes ----
            for sc in rp.sched:
                t0, cn = sc["tiles"], sc["n_tiles"]
                b = sc["bucket"]
                ne = cn * 128
                elg = sb2.tile([128, PASS2_CH, FATW], f32, tag="elg")
                erg = sb2.tile([128, PASS2_CH, FATW], f32, tag="erg")
                nc.gpsimd.dma_gather(
                    out_ap=elg[:, :cn, :], in_ap=elfat[:],
                    idxs_ap=loaded[(r, "zidx")][:, t0 * 8:(t0 + cn) * 8],
                    num_idxs=ne, num_idxs_reg=nreg(ne), elem_size=FATW)
                nc.gpsimd.dma_gather(
                    out_ap=erg[:, :cn, :],
                    in_ap=er_fat[r][b * BUCKET:min((b + 1) * BUCKET, fatN), :],
                    idxs_ap=loaded[(r, "didx")][:, t0 * 8:(t0 + cn) * 8],
                    num_idxs=ne, num_idxs_reg=nreg(ne), elem_size=FATW)
                ex = ex_all[:, t0:t0 + cn, :]
                nc.vector.tensor_tensor(out=ex, in0=elg[:, :cn, el_lo:el_hi],
                                        in1=erg[:, :cn, 0:4], op=AluOp.add)
                nc.vector.scalar_tensor_tensor(out=ex, in0=ex, scalar=0.2,
                                               in1=ex, op0=AluOp.mult, op1=AluOp.max)
                nc.scalar.activation(out=ex, in_=ex, func=Act.Exp)
                mk = loaded[("mask", r)][:, t0:t0 + cn]
                nc.vector.tensor_tensor(out=ex, in0=ex,
                                        in1=mk.unsqueeze(2).to_broadcast([128, cn, 4]),
                                        op=AluOp.mult)
                # per-tile one-hot merge of ex -> staging fat rows
                stg = sb2.tile([128, PASS2_CH, FATW], f32, tag="sstg")
                nc.vector.memset(stg[:], 0.0)
                for ti in range(cn):
                    S = build_S(t0 + ti)
                    pm = pst.tile([128, F], f32, space="PSUM", tag="pmg", bufs=2)
                    nc.tensor.matmul(out=pm[:, 0:4], lhsT=S[:],
                                     rhs=ex_all[:, t0 + ti, :], start=True, stop=True)
                    nc.vector.tensor_copy(out=stg[:, ti, 0:4], in_=pm[:, 0:4])
                nc.sync.dma_start(
                    out=s_send[(r, l)][:].rearrange("(t p) c -> p t c", p=128)[
                        :, sc["compact"] // 128:sc["compact"] // 128 + cn, :],
                    in_=stg[:, :cn, :])

            tc.strict_bb_all_engine_barrier()
            nc.gpsimd.collective_compute("AllToAll", AluOp.bypass, replica_groups=RG,
                                         ins=[s_send[(r, l)][:]], outs=[s_recv[(r, l)][:]])
            tc.strict_bb_all_engine_barrier()

            # ---- owner: merge s (one scatter_add per sender block) ----
            MT = rp.Mpad // 128
            for i in range(NCORES):
                tc.strict_bb_all_engine_barrier()
                for c0 in range(0, MT, PASS1_CH):
                    cn = min(PASS1_CH, MT - c0)
                    rin = sb2.tile([128, PASS1_CH, FATW], f32, tag="srecv")
                    nc.sync.dma_start(
                        out=rin[:, :cn, :],
                        in_=s_recv[(r, l)][:].rearrange("(t p) c -> p t c", p=128)[
                            :, i * MT + c0:i * MT + c0 + cn, :])
                    nc.gpsimd.dma_scatter_add(
                        out_ap=sown[(r, l)][:], in_ap=rin[:, :cn, :],
                        idxs_ap=loaded[(r, "midx")][:, (i * MT + c0) * 8:(i * MT + c0 + cn) * 8],
                        num_idxs=cn * 128, num_idxs_reg=nreg(cn * 128),
                        elem_size=FATW)

            npd = SP if r == "ww" else DP
            nt = npd // 128
            sfa = sown[(r, l)][:].rearrange("(t p) c -> p t c", p=128)
            iva = invs_loc[r][:].rearrange("(t p) c -> p t c", p=128)
            for n0 in range(0, nt, 16):
                nn = min(16, nt - n0)
                stile = sb.tile([128, 16, FATW], f32, tag="stile")
                nc.sync.dma_start(out=stile[:, :nn, :], in_=sfa[:, n0:n0 + nn, :])
                sv = stile[:, :nn, 0:4]
                nc.vector.tensor_scalar(out=sv, in0=sv, scalar1=1e-30, scalar2=None,
                                        op0=AluOp.max)
                nc.vector.reciprocal(out=sv, in_=sv)
                nc.sync.dma_start(out=iva[:, n0:n0 + nn, :], in_=stile[:, :nn, :])
            tc.strict_bb_all_engine_barrier()
            nc.gpsimd.collective_compute("AllGather", AluOp.bypass, replica_groups=RG,
                                         ins=[invs_loc[r][:]], outs=[invs_fat[r][:]])
            tc.strict_bb_all_engine_barrier()

            # ---- pass 2: alpha-weighted messages, per-tile merge, plain write ----
            ivN = invs_fat[r].shape[0]
            for sc in rp.sched:
                t0, cn = sc["tiles"], sc["n_tiles"]
                b = sc["bucket"]
                ne = cn * 128
                ivg = sb2.tile([128, PASS2_CH, FATW], f32, tag="ivg")
                nc.gpsimd.dma_gather(
                    out_ap=ivg[:, :cn, :],
                    in_ap=invs_fat[r][b * BUCKET:min((b + 1) * BUCKET, ivN), :],
                    idxs_ap=loaded[(r, "didx")][:, t0 * 8:(t0 + cn) * 8],
                    num_idxs=ne, num_idxs_reg=nreg(ne), elem_size=FATW)
                alp = sb2.tile([128, PASS2_CH, 4], f32, tag="alp")
                nc.vector.tensor_tensor(out=alp[:, :cn, :], in0=ex_all[:, t0:t0 + cn, :],
                                        in1=ivg[:, :cn, 0:4], op=AluOp.mult)
                zg = sb2.tile([128, PASS2_CH, HF], f32, tag="zg")
                nc.gpsimd.dma_gather(
                    out_ap=zg[:, :cn, :], in_ap=zt[:],
                    idxs_ap=loaded[(r, "zidx")][:, t0 * 8:(t0 + cn) * 8],
                    num_idxs=ne, num_idxs_reg=nreg(ne), elem_size=HF)
                mst = sb2.tile([128, PASS2_CH, F], f32, tag="mstg")
                for ti in range(cn):
                    msg = sb2.tile([128, F], f32, tag="msg")
                    for h in range(H):
                        zh = zg[:, ti, h * F:(h + 1) * F]
                        a = alp[:, ti, h:h + 1]
                        if h == 0:
                            nc.vector.tensor_scalar(out=msg[:], in0=zh, scalar1=a,
                                                    scalar2=None, op0=AluOp.mult)
                        else:
                            nc.vector.scalar_tensor_tensor(
                                out=msg[:], in0=zh, scalar=a, in1=msg[:],
                                op0=AluOp.mult, op1=AluOp.add)
                    S = build_S(t0 + ti)
                    pm = pst.tile([128, F], f32, space="PSUM", tag="pmg", bufs=2)
                    nc.tensor.matmul(out=pm[:], lhsT=S[:], rhs=msg[:],
                                     start=True, stop=True)
                    nc.vector.tensor_copy(out=mst[:, ti, :], in_=pm[:])
                nc.sync.dma_start(
                    out=m_send[(r, l)][:].rearrange("(t p) c -> p t c", p=128)[
                        :, sc["compact"] // 128:sc["compact"] // 128 + cn, :],
                    in_=mst[:, :cn, :])

            tc.strict_bb_all_engine_barrier()
            nc.gpsimd.collective_compute("AllToAll", AluOp.bypass, replica_groups=RG,
                                         ins=[m_send[(r, l)][:]], outs=[m_recv[(r, l)][:]])
            tc.strict_bb_all_engine_barrier()

            # ---- owner merge messages (one scatter_add per sender block) ----
            for i in range(NCORES):
                tc.strict_bb_all_engine_barrier()
                for c0 in range(0, MT, PASS2_CH):
                    cn = min(PASS2_CH, MT - c0)
                    rin = sb2.tile([128, PASS2_CH, F], f32, tag="mrecv")
                    nc.sync.dma_start(
                        out=rin[:, :cn, :],
                        in_=m_recv[(r, l)][:].rearrange("(t p) c -> p t c", p=128)[
                            :, i * MT + c0:i * MT + c0 + cn, :])
                    nc.gpsimd.dma_scatter_add(
                        out_ap=oacc[(r, l)][:], in_ap=rin[:, :cn, :],
                        idxs_ap=loaded[(r, "midx")][:, (i * MT + c0) * 8:(i * MT + c0 + cn) * 8],
                        num_idxs=cn * 128, num_idxs_reg=nreg(cn * 128), elem_size=F)

        # ================= full schedule =================
        dense(0)
        edge_phase("ww", 0)
        edge_phase("wd", 0)
        import os as _os
        if _os.environ.get("GAT_DEBUG"):
            dbg = {"d_z": z_tab["ww"], "d_elfat": elfat, "d_erfat": er_fat["ww"],
                   "d_ssend": s_send[("ww", 0)], "d_srecv": s_recv[("ww", 0)],
                   "d_sown": sown[("ww", 0)], "d_invs": invs_fat["ww"],
                   "d_msend": m_send[("ww", 0)], "d_oacc": oacc[("ww", 0)]}
            tc.strict_bb_all_engine_barrier()
            for nm, t in dbg.items():
                o = nc.dram_tensor(nm, list(t.shape), f32, kind="ExternalOutput")
                v = t[:].rearrange("a b -> (a b)").rearrange("(p n) -> p n", p=128)
                vo = o[:].rearrange("a b -> (a b)").rearrange("(p n) -> p n", p=128)
                for c0 in range(0, v.shape[1], 512):
                    w = min(512, v.shape[1] - c0)
                    tmp = sb.tile([128, 512], f32, tag="dbgcp")
                    nc.sync.dma_start(out=tmp[:, :w], in_=v[:, c0:c0 + w])
                    nc.sync.dma_start(out=vo[:, c0:c0 + w], in_=tmp[:, :w])
        dense(1)
        edge_phase("ww", 1)
        edge_phase("wd", 1)

        # final: y = relu(acc + bsum)
        for (r, out_t, nt) in (("ww", yw, WT), ("wd", yd, DT_)):
            for t in range(nt):
                rows = sb.tile([128, F], f32, tag="rows_out")
                nc.sync.dma_start(out=rows[:], in_=oacc[(r, 1)][t * 128:(t + 1) * 128, :])
                nc.vector.tensor_tensor(out=rows[:], in0=rows[:],
                                        in1=bsum_sb[(r, 1)][:], op=AluOp.add)
                nc.scalar.activation(out=rows[:], in_=rows[:], func=Act.Relu)
                nc.sync.dma_start(out=out_t[t * 128:(t + 1) * 128, :], in_=rows[:])

    nc.compile()
    return nc


# ---------------------------------------------------------------------------
# entry point
# ---------------------------------------------------------------------------

def _make_in_maps(cfg, ww, wd, xw, xd, params):
    in_maps = []
    for c in range(NCORES):
        m = {
            "ident": np.eye(128, dtype=np.float32),
            "xw0": xw[c], "xd0": xd[c],
            "W_ww0": params["W_ww0"], "W_wd0": params["W_wd0"],
            "W_ww1": params["W_ww1"], "W_wd1": params["W_wd1"],
            "elar0": params["elar0"], "elar1": params["elar1"],
        }
        for l in range(2):
            for r in ("ww", "wd"):
                m[f"bsum_{r}{l}"] = params[f"bsum_{r}{l}"].reshape(1, F)
        m["iota_row"] = np.broadcast_to(np.arange(128, dtype=np.float32), (128, 128)).copy()
        for r, rp in (("ww", ww), ("wd", wd)):
            m[f"{r}_zidx"] = rp.zidx[c]
            m[f"{r}_didx"] = rp.didx[c]
            m[f"{r}_slot"] = rp.slot[c]
            m[f"{r}_midx"] = rp.midx[c]
            m[f"{r}_mask"] = rp.mask[c]
        in_maps.append(m)
    return in_maps


def run(inputs, cfg=None, trace=False, mm_dt="float32r"):
    from concourse.bass_utils import run_bass_kernel_spmd
    if cfg is None:
        cfg = Cfg(80000, 16000, 200000, 200000)
    ww, wd, xw, xd, params = host_prep(cfg, inputs)
    nc = build_nc(cfg, ww, wd, mm_dt_name=mm_dt)
    in_maps = _make_in_maps(cfg, ww, wd, xw, xd, params)
    res = run_bass_kernel_spmd(nc, in_maps, list(range(NCORES)), trace=trace)
    outw = np.concatenate([res.results[c]["yw"][:cfg.SRC_SH] for c in range(NCORES)], 0)
    outd = np.concatenate([res.results[c]["yd"][:cfg.DOC_SH] for c in range(NCORES)], 0)
    return (outw, outd), res


def kernel(**inputs):
    (outw, outd), _ = run(inputs)
    return outw, outd
